# revision 40
# baseline (speedup 1.0000x reference)
"""CovPool kernel for 8 TRN2 NeuronCores.

reference semantics (B=32, N=16384, D=64):
    cov_b = (X_b - mean_b)^T (X_b - mean_b) / (N-1) + lam*I        (64x64)
    out   = sort(concat_b triu(cov_b)) reshaped to (B, 2080)

Device strategy (data parallel over batch, core c owns batches [4c, 4c+4)):
  per-core slab = 16 MiB, streamed once (DMA floor ~47 us @ 360 GB/s).
  - SP issues the big input DMAs (1 MiB each; nothing else on SP).
  - Activation casts each f32 buffer to bf16 (feeds the PE).
  - PE accumulates G = X^T X per batch: one [128,64]x[128,64] bf16 Gram
    matmul per 128-row chunk into PSUM.
  - DVE reduces per-buffer column partial sums (r-strided tensor_reduce),
    tree-adds them, then PE folds partitions via a ones-matmul and applies
    the rank-1 mean correction -s s^T / N into PSUM.
  - DVE scales PSUM to cov + lam*I into a shared [64, 4*64] tile; one
    merged output DMA (issued from DVE) writes all 4 covs.
  - host: triu extract + global sort (tiny: 32x2080 elements).
  The final batch's stream is tapered (smaller trailing DMAs) to cut the
  pipeline drain after the last transfer.
"""

import sys

sys.path.insert(0, "/opt/trn_rl_repo")

import numpy as np

from concourse import bacc, mybir
from concourse.tile import TileContext
from concourse.bass_utils import run_bass_kernel_spmd

B, N, D = 32, 16384, 64
NCORES = 8
BPC = B // NCORES  # batches per core
LAMBDA = 0.01
D_OUT = D * (D + 1) // 2  # 2080

MAX_R = 32                  # max rows per partition per stream buffer
MAX_FREE = MAX_R * D        # 2048 f32 per partition = 8 KB (1 MiB DMA)
NSTREAM = 6                 # stream ring depth
SUMW = 512                  # ones-matmul quarter width (one PSUM bank row)

# per-batch segment plans, in rows-per-partition units (sum = 128 per batch
# since 128 partitions * 128 r-units = 16384 rows)
FULL_PLAN = [32, 32, 32, 32]
TAPER_PLAN = [32, 32, 24, 16, 8, 8, 4, 2, 2]  # last batch: drain taper

f32 = mybir.dt.float32
bf16 = mybir.dt.bfloat16


def _emit_body(tc, nc, x, out, stream, work_pool, psum_pool,
               variant="full"):
    """One full covariance pass over this core's BPC batches.

    Each batch's tail (mean fold + rank-1 correction + cov scale) is
    deferred into the NEXT batch's stream so its inputs are long ready by
    the time the in-order PE/DVE queues reach it — no sequencer stalls.
    """
    di = 0  # global stream-slot counter
    stream_f32, stream_bf = stream
    gram = out
    xf = x.rearrange("b n d -> b (n d)")  # flat per-batch view

    def tail(b, psum, partials, nseg):
        # Ship raw Gram + per-partition column sums; host does the
        # partition fold + rank-1 mean correction. Two parallel chains:
        # Act copies the Gram rows from PSUM, DVE folds the per-segment
        # partials over t.
        gram_sb = work_pool.tile([128, 2 * D], f32, tag=f"gram{b % 2}")
        nc.scalar.copy(gram_sb[0:D, 0:D], psum[0:D, 0:D])
        nc.vector.tensor_reduce(
            out=gram_sb[:, D:2 * D],
            in_=partials[:, 0:nseg * D].rearrange("p (t d) -> p d t", d=D),
            axis=mybir.AxisListType.X, op=mybir.AluOpType.add,
        )
        if b == BPC - 1:
            # final batch: SP's input queue is drained, shortest DGE path
            nc.sync.dma_start(gram[b], gram_sb[:])
        else:
            # mid-stream: issue from Pool (SWDGE) so neither SP's input
            # stream nor Act's cast queue is ever held up by this wait
            nc.gpsimd.dma_start(gram[b], gram_sb[:])

    # ablation switches (timing-only variants; output garbage except full)
    do_cast = variant in ("full", "no_mm", "no_red")
    do_gram = variant in ("full", "no_red", "no_cast")
    do_red = variant in ("full", "no_cast")
    do_tail = variant in ("full", "no_cast")

    prev = None  # (b, psum, partials, nseg) awaiting tail emission
    for b in range(BPC):
        plan = TAPER_PLAN if b == BPC - 1 else FULL_PLAN
        psum = psum_pool.tile([D + 1, 512], f32, tag=f"acc{b % BPC}")
        partials = work_pool.tile([128, len(plan) * D], f32,
                                  tag=f"part{b % 2}_{len(plan)}")
        last = (len(plan) - 1, plan[-1] - 1)
        row0 = 0  # running row offset
        for t, r_per_part in enumerate(plan):
            free = r_per_part * D
            nelem = 128 * r_per_part * D
            buf = stream_f32[di % NSTREAM]
            bbuf = stream_bf[di % NSTREAM]
            di += 1
            nc.sync.dma_start(
                buf[:, 0:free],
                xf[b, row0 * D:row0 * D + nelem]
                .rearrange("(p f) -> p f", p=128),
            )
            row0 += 128 * r_per_part
            if do_cast:
                # fp32 -> bf16 cast for the PE, all on Act (DVE is busy
                # with the column-sum reduces)
                nc.scalar.copy(bbuf[:, 0:free], buf[:, 0:free])
            if do_gram:
                for r in range(r_per_part):
                    nc.tensor.matmul(
                        psum[0:D, 0:D], bbuf[:, r * D:(r + 1) * D],
                        bbuf[:, r * D:(r + 1) * D],
                        start=(t == 0 and r == 0), stop=((t, r) == last),
                    )
            if do_red:
                # per-segment column partial sums on DVE straight off the
                # f32 stream (keeps the PE at Gram-only so it never
                # exceeds the per-segment budget even when throttled)
                nc.vector.tensor_reduce(
                    out=partials[:, t * D:(t + 1) * D],
                    in_=buf[:, 0:free].rearrange("p (r d) -> p d r", d=D),
                    axis=mybir.AxisListType.X, op=mybir.AluOpType.add,
                )
            if do_tail and t == 0 and prev is not None:
                tail(*prev)
                prev = None
        prev = (b, psum, partials, len(plan))
    if do_tail:
        tail(*prev)
    else:
        # timing-only variants: emit garbage outputs so the NEFF has writers
        gram_sb = work_pool.tile([128, 2 * D], f32, tag="gram0")
        nc.vector.memset(gram_sb[:], 0.5)
        for b in range(BPC):
            nc.scalar.dma_start(gram[b], gram_sb[:])


def build_cov_kernel(bench_reps=None, variant="full"):
    assert variant in ("full", "dma_only", "no_mm", "no_red", "no_cast")
    nc = bacc.Bacc("TRN2", target_bir_lowering=False, debug=False,
                   num_devices=NCORES)
    x = nc.dram_tensor("x", [BPC, N, D], f32, kind="ExternalInput")
    gram = nc.dram_tensor("gram", [BPC, 128, 2 * D], f32,
                          kind="ExternalOutput")

    with TileContext(nc) as tc:
        with (
            tc.tile_pool(name="stream", bufs=1) as stream_pool,
            tc.tile_pool(name="work", bufs=2) as work_pool,
            tc.tile_pool(name="psum", bufs=1, space="PSUM") as psum_pool,
        ):
            stream_f32 = [
                stream_pool.tile([128, MAX_FREE], f32,
                                 tag=f"stream{i}", name=f"stream{i}")
                for i in range(NSTREAM)
            ]
            stream_bf = [
                stream_pool.tile([128, MAX_FREE], bf16,
                                 tag=f"streambf{i}", name=f"streambf{i}")
                for i in range(NSTREAM)
            ]
            stream = (stream_f32, stream_bf)
            if variant == "no_cast":
                for t_ in stream_bf:
                    nc.vector.memset(t_[:], 0.5)

            def body():
                _emit_body(tc, nc, x, gram, stream,
                           work_pool, psum_pool, variant=variant)

            if bench_reps is None:
                body()
            else:
                with tc.For_i(0, bench_reps, 1):
                    body()

    nc.compile()
    return nc


_NC_CACHE = {}


def _get_kernel():
    if "nc" not in _NC_CACHE:
        _NC_CACHE["nc"] = build_cov_kernel()
    return _NC_CACHE["nc"]


def make_in_maps(x_full: np.ndarray):
    return [
        {"x": np.ascontiguousarray(x_full[c * BPC:(c + 1) * BPC])}
        for c in range(NCORES)
    ]


def postprocess(results):
    """results: list of per-core out dicts -> final (B, D_OUT) array.

    Device ships raw Gram G = X^T X and per-partition column sums; the
    tiny mean correction / scale / +lam*I runs here.
    """
    raw = np.concatenate([results[c]["gram"] for c in range(NCORES)],
                         axis=0)  # (B, 128, 2D)
    G = raw[:, 0:D, 0:D]
    s = raw[:, :, D:2 * D].sum(axis=1)  # (B, D) partition fold
    cov = (G - s[:, :, None] * s[:, None, :] / N) / (N - 1)
    cov += LAMBDA * np.eye(D, dtype=np.float32)[None]
    iu, ju = np.triu_indices(D)
    tri = cov[:, iu, ju]  # (B, D_OUT)
    return np.sort(tri.reshape(-1)).reshape(B, D_OUT).astype(np.float32)


def run_device(x_full: np.ndarray):
    nc = _get_kernel()
    res = run_bass_kernel_spmd(nc, make_in_maps(x_full),
                               core_ids=list(range(NCORES)))
    return res.results


def kernel(x: np.ndarray) -> np.ndarray:
    x = np.asarray(x, dtype=np.float32)
    return postprocess(run_device(x))


if __name__ == "__main__":
    rng = np.random.default_rng(0)
    xt = rng.standard_normal((B, N, D), dtype=np.float32)
    o = kernel(xt)
    print("kernel out shape:", o.shape, o.dtype)


# revision 50
# speedup vs baseline: 1.1565x; 1.1565x over previous
"""CovPool kernel for 8 TRN2 NeuronCores.

reference semantics (B=32, N=16384, D=64):
    cov_b = (X_b - mean_b)^T (X_b - mean_b) / (N-1) + lam*I        (64x64)
    out   = sort(concat_b triu(cov_b)) reshaped to (B, 2080)

Device strategy (data parallel over batch, core c owns batches [4c, 4c+4)):
  per-core slab = 16 MiB, streamed once (DMA floor ~47 us @ 360 GB/s).
  - SP issues the big input DMAs (1 MiB each; nothing else on SP).
  - Activation casts each f32 buffer to bf16 (feeds the PE).
  - PE accumulates G = X^T X per batch: one [128,64]x[128,64] bf16 Gram
    matmul per 128-row chunk into PSUM.
  - DVE reduces per-buffer column partial sums (r-strided tensor_reduce),
    tree-adds them, then PE folds partitions via a ones-matmul and applies
    the rank-1 mean correction -s s^T / N into PSUM.
  - DVE scales PSUM to cov + lam*I into a shared [64, 4*64] tile; one
    merged output DMA (issued from DVE) writes all 4 covs.
  - host: triu extract + global sort (tiny: 32x2080 elements).
  The final batch's stream is tapered (smaller trailing DMAs) to cut the
  pipeline drain after the last transfer.
"""

import sys

sys.path.insert(0, "/opt/trn_rl_repo")

import numpy as np

from concourse import bacc, mybir
from concourse.tile import TileContext
from concourse.bass_utils import run_bass_kernel_spmd

B, N, D = 32, 16384, 64
NCORES = 8
BPC = B // NCORES  # batches per core
LAMBDA = 0.01
D_OUT = D * (D + 1) // 2  # 2080

MAX_R = 32                  # max rows per partition per stream buffer
MAX_FREE = MAX_R * D        # 2048 f32 per partition = 8 KB (1 MiB DMA)
NSTREAM = 6                 # stream ring depth
SUMW = 512                  # ones-matmul quarter width (one PSUM bank row)

# per-batch segment plans, in rows-per-partition units (sum = 128 per batch
# since 128 partitions * 128 r-units = 16384 rows)
FULL_PLAN = [32, 32, 32, 32]
TAPER_PLAN = [32, 32, 24, 16, 8, 8, 4, 2, 2]  # last batch: drain taper

f32 = mybir.dt.float32
bf16 = mybir.dt.bfloat16


def _emit_body(tc, nc, x, out, stream, work_pool, psum_pool,
               variant="full"):
    """One full covariance pass over this core's BPC batches.

    Each batch's tail (mean fold + rank-1 correction + cov scale) is
    deferred into the NEXT batch's stream so its inputs are long ready by
    the time the in-order PE/DVE queues reach it — no sequencer stalls.
    """
    di = 0  # global stream-slot counter
    stream_f32, stream_bf = stream
    gram, ones_col = out
    xf = x.rearrange("b n d -> b (n d)")  # flat per-batch view

    def tail(b, psum):
        # Ship raw Gram + quartered column sums (psum row D); host folds
        # the quarters + rank-1 mean correction. Two parallel chains:
        # Act copies the Gram rows from PSUM, DVE folds the sum row.
        gram_sb = work_pool.tile([D + 1, D], f32, tag=f"gram{b % 2}")
        nc.scalar.copy(gram_sb[0:D, 0:D], psum[0:D, 0:D])
        nc.vector.tensor_reduce(
            out=gram_sb[D:D + 1, 0:D],
            in_=psum[D:D + 1, 0:SUMW].rearrange("p (q d) -> p d q", d=D),
            axis=mybir.AxisListType.X, op=mybir.AluOpType.add,
        )
        if b == BPC - 1:
            # final batch: SP's input queue is drained, shortest DGE path
            nc.sync.dma_start(gram[b], gram_sb[:])
        else:
            # mid-stream: issue from Pool (SWDGE) so neither SP's input
            # stream nor Act's cast queue is ever held up by this wait
            nc.gpsimd.dma_start(gram[b], gram_sb[:])

    # ablation switches (timing-only variants; output garbage except full)
    do_cast = variant in ("full", "no_mm", "no_red")
    do_gram = variant in ("full", "no_red", "no_cast")
    do_ones = variant in ("full", "no_cast")
    do_tail = variant in ("full", "no_cast")
    dma2 = variant == "dma_only2"

    def emit_mms(pair, psum, gstate, qstate):
        """PE work for a pair of segments, REVERSED: the first matmul
        waits on the pair's LAST cast, so by the time the in-order PE
        reaches it every operand is ready and the whole pair runs as one
        contiguous burst (keeps the tensor engine out of its low-clock
        p-state, which it falls into when matmuls trickle in with idle
        gaps between segments). start/stop flags follow EMISSION order
        (the Gram sum is chunk-order independent)."""
        for t, r_per_part, bbuf in reversed(pair):
            free = r_per_part * D
            if do_gram:
                for r in range(r_per_part):
                    nc.tensor.matmul(
                        psum[0:D, 0:D], bbuf[:, r * D:(r + 1) * D],
                        bbuf[:, r * D:(r + 1) * D],
                        start=gstate[0], stop=(gstate[1] == 1),
                    )
                    gstate[0] = False
                    gstate[1] -= 1
            if do_ones:
                for q0 in range(0, free, SUMW):
                    w = min(SUMW, free - q0)
                    nc.tensor.matmul(
                        psum[D:D + 1, 0:w], ones_col[:],
                        bbuf[:, q0:q0 + w],
                        start=qstate[0], stop=(qstate[1] == 1),
                    )
                    qstate[0] = False
                    qstate[1] -= 1

    prev = None  # (b, psum) awaiting tail emission
    for b in range(BPC):
        plan = TAPER_PLAN if b == BPC - 1 else FULL_PLAN
        psum = psum_pool.tile([D + 1, 512], f32, tag=f"acc{b % BPC}")
        row0 = 0  # running row offset
        # accumulation bookkeeping (emission order): [need_start, left]
        gstate = [True, sum(plan)]
        qstate = [True, sum(-(-r * D // SUMW) for r in plan)]
        pair = []
        for t, r_per_part in enumerate(plan):
            free = r_per_part * D
            nelem = 128 * r_per_part * D
            buf = stream_f32[di % NSTREAM]
            bbuf = stream_bf[di % NSTREAM]
            eng = nc.scalar if (dma2 and di % 2) else nc.sync
            di += 1
            eng.dma_start(
                buf[:, 0:free],
                xf[b, row0 * D:row0 * D + nelem]
                .rearrange("(p f) -> p f", p=128),
            )
            row0 += 128 * r_per_part
            if do_cast:
                # fp32 -> bf16 cast, alternating Act / DVE
                if di % 2 == 1:
                    nc.scalar.copy(bbuf[:, 0:free], buf[:, 0:free])
                else:
                    nc.vector.tensor_copy(bbuf[:, 0:free], buf[:, 0:free])
            pair.append((t, r_per_part, bbuf))
            if len(pair) == 2 or t == len(plan) - 1:
                emit_mms(pair, psum, gstate, qstate)
                pair = []
            if do_tail and t == 0 and prev is not None:
                tail(*prev)
                prev = None
        prev = (b, psum)
    if do_tail:
        tail(*prev)
    else:
        # timing-only variants: emit garbage outputs so the NEFF has writers
        gram_sb = work_pool.tile([D + 1, D], f32, tag="gram0")
        nc.vector.memset(gram_sb[:], 0.5)
        for b in range(BPC):
            nc.scalar.dma_start(gram[b], gram_sb[:])


def build_cov_kernel(bench_reps=None, variant="full"):
    assert variant in ("full", "dma_only", "dma_only2", "no_mm", "no_red",
                       "no_cast")
    nc = bacc.Bacc("TRN2", target_bir_lowering=False, debug=False,
                   num_devices=NCORES)
    x = nc.dram_tensor("x", [BPC, N, D], f32, kind="ExternalInput")
    gram = nc.dram_tensor("gram", [BPC, D + 1, D], f32,
                          kind="ExternalOutput")

    with TileContext(nc) as tc:
        with (
            tc.tile_pool(name="stream", bufs=1) as stream_pool,
            tc.tile_pool(name="const", bufs=1) as const_pool,
            tc.tile_pool(name="work", bufs=2) as work_pool,
            tc.tile_pool(name="psum", bufs=1, space="PSUM") as psum_pool,
        ):
            ones_col = const_pool.tile([128, 1], bf16, tag="ones")
            nc.vector.memset(ones_col[:], 1.0)
            stream_f32 = [
                stream_pool.tile([128, MAX_FREE], f32,
                                 tag=f"stream{i}", name=f"stream{i}")
                for i in range(NSTREAM)
            ]
            stream_bf = [
                stream_pool.tile([128, MAX_FREE], bf16,
                                 tag=f"streambf{i}", name=f"streambf{i}")
                for i in range(NSTREAM)
            ]
            stream = (stream_f32, stream_bf)
            if variant == "no_cast":
                for t_ in stream_bf:
                    nc.vector.memset(t_[:], 0.5)

            def body():
                _emit_body(tc, nc, x, (gram, ones_col), stream,
                           work_pool, psum_pool, variant=variant)

            if bench_reps is None:
                body()
            else:
                with tc.For_i(0, bench_reps, 1):
                    body()

    nc.compile()
    return nc


_NC_CACHE = {}


def _get_kernel():
    if "nc" not in _NC_CACHE:
        _NC_CACHE["nc"] = build_cov_kernel()
    return _NC_CACHE["nc"]


def make_in_maps(x_full: np.ndarray):
    return [
        {"x": np.ascontiguousarray(x_full[c * BPC:(c + 1) * BPC])}
        for c in range(NCORES)
    ]


def postprocess(results):
    """results: list of per-core out dicts -> final (B, D_OUT) array.

    Device ships raw Gram G = X^T X and per-partition column sums; the
    tiny mean correction / scale / +lam*I runs here.
    """
    raw = np.concatenate([results[c]["gram"] for c in range(NCORES)],
                         axis=0)  # (B, D+1, D)
    G = raw[:, 0:D, :]
    s = raw[:, D, :]  # (B, D)
    cov = (G - s[:, :, None] * s[:, None, :] / N) / (N - 1)
    cov += LAMBDA * np.eye(D, dtype=np.float32)[None]
    iu, ju = np.triu_indices(D)
    tri = cov[:, iu, ju]  # (B, D_OUT)
    return np.sort(tri.reshape(-1)).reshape(B, D_OUT).astype(np.float32)


def run_device(x_full: np.ndarray):
    nc = _get_kernel()
    res = run_bass_kernel_spmd(nc, make_in_maps(x_full),
                               core_ids=list(range(NCORES)))
    return res.results


def kernel(x: np.ndarray) -> np.ndarray:
    x = np.asarray(x, dtype=np.float32)
    return postprocess(run_device(x))


if __name__ == "__main__":
    rng = np.random.default_rng(0)
    xt = rng.standard_normal((B, N, D), dtype=np.float32)
    o = kernel(xt)
    print("kernel out shape:", o.shape, o.dtype)
